# revision 16
# baseline (speedup 1.0000x reference)
"""Masked dot-product attention (B=16, Lq=Lk=2048, d=64) on 8 TRN2 NeuronCores.

Distribution
------------
Attention rows are independent, so work is split into 64 units = (batch,
512-query chunk). Unit cost = ceil(valid_len/128) k-tiles; fully-masked
k-tiles contribute exactly zero and are skipped. Units are sorted by cost
(ascending) and snake-assigned to 8 slots x 8 cores; each slot's tile
count is the max within the slot, so all 8 cores run ONE shared SPMD
program (per-core differences live only in the staged data).

Device math per unit (S^T formulation; softmax over the partition axis):
    s_t[k, q]  = (K^T_t weights) @ Q^T           (PE, bf16 in / f32 PSUM)
    p_t[k, q]  = exp(0.125 * s_t)                (ACT, PSUM->SBUF, bf16)
    pv[v, q]  += V'_t^T @ p_t                    (PE, accumulate over t)
where V'_t = [V rows | ones], with rows >= valid_len zeroed on the host —
this applies the key mask AND computes the softmax denominator l = pv[64]
inside the same matmul. No row-max subtraction is needed: scores are
O(+-10) (exact softmax shift-invariance; masked lanes match the
reference's exp(-1e6)->0). valid_len == 0 reproduces jax's uniform
softmax by zeroing Q (s = 0 -> p = 1) and leaving V' unmasked.

The device returns RAW [pv[0:64] | l] per slot; the final o = pv/l
division happens vectorized on the host. This removes the entire device
epilogue (reciprocal + cross-partition broadcast chains) that previously
dominated the kernel tail.

Performance notes (measured on this axon-tunneled TRN2):
- The PE streams 1 column/cycle regardless of dtype; the clock ramps
  ~0.8 -> 1.3 GHz once HAM grants full credits, then halves if the
  kernel runs long. Shorter wall time = more of the kernel at full clock.
- kv, Q^T and p are staged/produced in bf16: halves HBM traffic (DMA
  prefetch completes by ~20us, which empirically pulls the HAM full-
  clock grant much earlier) at ~3.3e-3 l2 rel err (gate is 2e-2).
  PSUM accumulation and the staged output stay f32.
- S matmuls are K=64 (half the PE rows), so tiles are processed in
  PAIRS: even tile's K^T sits at SBUF partitions 0:64, odd tile's at
  64:127 (host staging), Q^T is loaded twice into both partition halves
  (one DRAM copy, two DMAs); the pair's two matmuls run concurrently in
  disjoint PE row halves (~494ns/pair vs ~780ns serial). GROUP=2 makes
  EVERY full group a pair (the old GROUP=3 left one unpaired tile per
  group).
- kv is staged as 4-tile chunks [V'0|V'1|V'2|V'3|K01|K23] with each
  pair's K^T packed into one fully-used [128,128] block — no zero
  padding in the K region (the old layout DMA'd 33% zeros), 2KB+
  partition lines, and only ~15 kv DMA issues per core.
- DMA issues are spread across queues: kv on Sync, Q^T on Vector,
  outputs on GpSimd — issue serialization (~0.6us/DMA) never gates
  the first matmul.
- The PE instruction queue is in-order, so PV matmuls of pair-group g
  are emitted after the S matmuls of group g+2 (two-group software
  pipeline); the PE never stalls on the exp.
- PSUM budget: s tiles 2 banks x 3 bufs + pv 1 bank x 2 bufs = 8 banks.
- Every TPB instruction may carry at most ONE sync wait on this walrus;
  split_multi_waits() post-processes the scheduled program into
  single-wait form with wait-carrying NoOps.
"""
import numpy as np
import ml_dtypes

import concourse.bass as bass
import concourse.mybir as mybir
import concourse.tile as tile
from concourse.bass_utils import run_bass_kernel_spmd


def split_multi_waits(nc):
    """TRN2 TPB instructions encode a single sync-wait slot. Tile's
    add_semaphores can emit several waits on one instruction (and the
    kernel-tail drain aggregates one per live proc), which walrus rejects
    ("Too many sync wait commands"). Rewrite every instruction carrying
    k>1 waits into (k-1) same-engine NoOps carrying one wait each."""
    for fn in nc.m.functions:
        for bb in fn.blocks:
            new = []
            for inst in bb.instructions:
                si = inst.sync_info
                ow = list(si.on_wait) if si else []
                if len(ow) > 1:
                    for jj, w in enumerate(ow[:-1]):
                        nop = mybir.InstNoOp(
                            name=f"{inst.name}_sw{jj}", ins=[], outs=[])
                        nop.engine = inst.engine
                        nop.sync_info = mybir.SyncInfo(
                            on_wait=[w], on_update=[])
                        new.append(nop)
                    inst.sync_info = mybir.SyncInfo(
                        on_wait=[ow[-1]], on_update=list(si.on_update))
                new.append(inst)
            bb.instructions = new

F32 = mybir.dt.float32
F32R = mybir.dt.float32r
BF16 = mybir.dt.bfloat16

B, L, D = 16, 2048, 64
QC = 512                 # query-chunk (free dim of both matmuls)
NQCHUNK = L // QC        # 4 chunks per batch
KT = 128                 # k rows per tile
N_CORES = 8
N_SLOTS = (B * NQCHUNK) // N_CORES   # 8 units per core
GROUP = 2                # k-tiles per pair-group (1 PSUM s-tile = 2 banks)
CHUNK = 4                # k-tiles per kv DMA chunk (2 pair-groups)
W_CHUNK = CHUNK * 65 + (CHUNK // 2) * 128   # 516 f32 per partition row


def _chunk_width(u):
    """Staged f32 width of a kv chunk holding u (1..4) tiles: CHUNK V'
    blocks of 65 (trailing ones zero-padded) then ceil(u/2) packed K^T
    blocks of 128 at the fixed CHUNK*65 base."""
    return CHUNK * 65 + ((u + 1) // 2) * 128


def _schedule(valid_lens):
    """Snake-assign 64 units to 8 slots x 8 cores. Returns (N_list, assign)
    where assign[core][slot] = (batch, qchunk) and N_list[slot] = tile
    count every core runs for that slot."""
    evl = np.where(valid_lens > 0, valid_lens, L).astype(np.int64)
    cost = np.ceil(evl / KT).astype(np.int64)        # per batch
    units = [(int(cost[b]), b, qc) for b in range(B) for qc in range(NQCHUNK)]
    units.sort(key=lambda t: (t[0], t[1], t[2]))
    N_list = []
    assign = [[None] * N_SLOTS for _ in range(N_CORES)]
    for j in range(N_SLOTS):
        grp = units[j * N_CORES:(j + 1) * N_CORES]
        N_list.append(grp[-1][0])
        for c in range(N_CORES):
            _, b, qc = grp[c]
            assign[c][j] = (b, qc)
    return N_list, assign


_PROGRAM_CACHE = {}


def _build_program(N_list):
    key = tuple(N_list)
    if key in _PROGRAM_CACHE:
        return _PROGRAM_CACHE[key]
    # kv staged chunk-major: chunk = up to 4 tiles in one DMA
    n_chunks = [int(np.ceil(n / CHUNK)) for n in N_list]
    TC = int(sum(n_chunks))
    coff = [0]
    for g in n_chunks:
        coff.append(coff[-1] + g)

    nc = bass.Bass()
    kv_d = nc.declare_dram_parameter("kv", [TC, KT, W_CHUNK], BF16,
                                     isOutput=False)
    qT_d = nc.declare_dram_parameter("qT", [N_SLOTS, D, QC], BF16,
                                     isOutput=False)
    o_d = nc.declare_dram_parameter("o", [N_SLOTS, 65, QC], F32,
                                    isOutput=True)

    with tile.TileContext(nc) as tc:
        with (
            tc.tile_pool(name="kv_pool", bufs=15) as kv_pool,
            tc.tile_pool(name="q_pool", bufs=1) as q_pool,
            tc.tile_pool(name="p_pool", bufs=5) as p_pool,
            tc.tile_pool(name="ep_pool", bufs=3) as ep_pool,
            tc.tile_pool(name="warm_pool", bufs=1) as warm_pool,
            tc.tile_pool(name="s_pool", bufs=3, space="PSUM") as s_pool,
            tc.tile_pool(name="pv_pool", bufs=2, space="PSUM") as pv_pool,
        ):
            # ACT exp-table warm-up: overlap the one-time table load with
            # the first DMAs instead of stalling the first real group.
            warm = warm_pool.tile([1, 1], F32)
            nc.vector.memset(warm, 0.0)
            nc.scalar.activation(warm, warm, mybir.ActivationFunctionType.Exp)

            # All 8 units' Q^T in one resident tile. One DRAM copy, two
            # half-height DMAs per slot (both halves identical) so the S
            # pair's even/odd matmuls find Q^T in both partition halves.
            # Two parallel DMAs also double the in-flight rate for slot 0.
            qt_all = q_pool.tile([KT, N_SLOTS, QC], BF16)

            def load_qt(j):
                src = bass.AP(tensor=qT_d, offset=j * D * QC,
                              ap=[[QC, D], [1, QC]])
                nc.gpsimd.dma_start(out=qt_all[0:D, j, :], in_=src)
                nc.gpsimd.dma_start(out=qt_all[D:KT, j, :], in_=src)

            # prefetch ALL slots' Q^T upfront: the 16 small DMAs issue
            # early on the GpSimd queue (otherwise idle until epilogues),
            # front-loading HBM traffic so the DMA system goes quiet as
            # soon as possible
            for j in range(N_SLOTS):
                load_qt(j)

            # two-group-deep software pipeline: PV matmuls of pair-group g
            # are emitted after the S matmuls of group g+2, so the in-order
            # PE queue never waits on the exp.
            PIPE_DEPTH = 2
            pending = []       # [(pv, pvsl, p, t0, n, g, j), ...]

            def flush_one():
                if not pending:
                    return
                pv, pvsl, p, t0, n, g, _ = pending.pop(0)
                for i in range(g):
                    nc.tensor.matmul(pv[0:65, :], lhsT=pvsl[i],
                                     rhs=p[:, i * QC:(i + 1) * QC],
                                     start=(t0 + i == 0),
                                     stop=(t0 + i == n - 1))

            epilogues = []     # (j, pv) awaiting copy-out (after PV flush)

            def emit_epilogues():
                # a slot's copy-out may only run once every PV group of its
                # unit has been flushed (program order defines semantics);
                # it frees the PSUM accumulator bank
                while epilogues and (not pending
                                     or epilogues[0][0] < pending[0][6]):
                    j, pv = epilogues.pop(0)
                    pvc = ep_pool.tile([65, QC], F32, tag="pvc")
                    nc.vector.tensor_copy(pvc, pv[0:65, :])
                    if j == N_SLOTS - 1:
                        # the final write gates the end-of-kernel drain:
                        # two halves on the (by now idle) Scalar queue get
                        # 2x in-flight rate and skip the GpSimd queue's
                        # issue backlog
                        nc.scalar.dma_start(out=o_d[j][0:33, :],
                                            in_=pvc[0:33, :])
                        nc.scalar.dma_start(out=o_d[j][33:65, :],
                                            in_=pvc[33:65, :])
                    else:
                        nc.gpsimd.dma_start(out=o_d[j], in_=pvc)

            for j in range(N_SLOTS):
                n = N_list[j]
                pv = pv_pool.tile([KT, QC], F32, tag="pv")
                t = 0
                kvc = None
                for t in range(0, n, GROUP):
                    g = min(GROUP, n - t)
                    if t % CHUNK == 0:
                        u = min(CHUNK, n - t)
                        w = _chunk_width(u)
                        cidx = coff[j] + t // CHUNK
                        kvc = kv_pool.tile([KT, W_CHUNK], BF16, tag="kv")
                        nc.sync.dma_start(out=kvc[:, 0:w],
                                          in_=kv_d[cidx][:, 0:w])
                        kbase = CHUNK * 65
                    s = s_pool.tile([KT, GROUP * QC], F32, tag="s")
                    pvsl = []
                    for i in range(g):
                        ic = (t % CHUNK) + i          # tile index in chunk
                        lo = D * (ic % 2)
                        ko = kbase + (ic // 2) * 128  # packed K^T block
                        nc.tensor.matmul(s[:, i * QC:(i + 1) * QC],
                                         lhsT=kvc[lo:lo + D, ko:ko + 128],
                                         rhs=qt_all[lo:lo + D, j, :],
                                         start=True, stop=True)
                        pvsl.append(kvc[:, ic * 65:(ic + 1) * 65])
                    p = p_pool.tile([KT, GROUP * QC], BF16, tag="p")
                    nc.scalar.activation(p[:, 0:g * QC], s[:, 0:g * QC],
                                         mybir.ActivationFunctionType.Exp,
                                         scale=0.125)
                    if len(pending) >= PIPE_DEPTH:
                        flush_one()
                        emit_epilogues()
                    pending.append((pv, pvsl, p, t, n, g, j))
                epilogues.append((j, pv))
            while pending:
                flush_one()
                emit_epilogues()
            emit_epilogues()

    split_multi_waits(nc)
    _PROGRAM_CACHE[key] = (nc, coff)
    return nc, coff


def _stage_inputs(queries, keys, values, valid_lens, N_list, assign, coff):
    evl = np.where(valid_lens > 0, valid_lens, L).astype(np.int64)
    zero_q = valid_lens <= 0
    TC = coff[-1]

    # Per-batch precomputed host tensors
    kTT = np.ascontiguousarray(keys.transpose(0, 2, 1))        # [B, D, L]
    vmask = (np.arange(L)[None, :] < evl[:, None])             # [B, L]
    vp = np.concatenate(
        [values, np.ones((B, L, 1), np.float32)], axis=2)      # [B, L, 65]
    vp = vp * vmask[:, :, None].astype(np.float32)

    in_maps = []
    for c in range(N_CORES):
        # chunk-major: kv[ch] = [V'0|V'1|V'2|V'3|K01|K23]; pair (2i,2i+1)
        # K^T packed into one [128,128] block (even rows 0:64, odd 64:128)
        kv = np.zeros((TC, KT, W_CHUNK), ml_dtypes.bfloat16)
        qT = np.zeros((N_SLOTS, D, QC), ml_dtypes.bfloat16)
        for j in range(N_SLOTS):
            b, qc = assign[c][j]
            n_real = int(np.ceil(evl[b] / KT))
            if not zero_q[b]:
                qT[j] = queries[b, qc * QC:(qc + 1) * QC, :].T
            n = min(n_real, N_list[j])
            nch = coff[j + 1] - coff[j]
            ntp = nch * CHUNK                      # padded tile count
            vt = np.zeros((ntp, KT, 65), np.float32)
            kt = np.zeros((ntp, D, KT), np.float32)
            vt[0:n] = vp[b, 0:n * KT].reshape(n, KT, 65)
            kt[0:n] = kTT[b, :, 0:n * KT].reshape(D, n, KT).transpose(1, 0, 2)
            sl = slice(coff[j], coff[j + 1])
            # V' blocks at i*65
            kv[sl, :, 0:CHUNK * 65] = (
                vt.reshape(nch, CHUNK, KT, 65)
                  .transpose(0, 2, 1, 3).reshape(nch, KT, CHUNK * 65))
            # packed K^T: pair p of chunk -> cols CHUNK*65 + p*128
            ktg = kt.reshape(nch, CHUNK // 2, 2, D, KT)
            kb = CHUNK * 65
            for pidx in range(CHUNK // 2):
                kv[sl, 0:D, kb + pidx * 128:kb + pidx * 128 + KT] = \
                    ktg[:, pidx, 0]
                kv[sl, D:, kb + pidx * 128:kb + pidx * 128 + KT] = \
                    ktg[:, pidx, 1]
        in_maps.append({"kv": kv, "qT": qT})
    return in_maps


def _gather(results, assign):
    out = np.empty((B, L, D), np.float32)
    for c in range(N_CORES):
        o = results[c]["o"]                       # [N_SLOTS, 65, QC]
        num = o[:, 0:D, :]
        den = o[:, D:D + 1, :]
        r = num / den                             # [N_SLOTS, D, QC]
        for j in range(N_SLOTS):
            b, qc = assign[c][j]
            out[b, qc * QC:(qc + 1) * QC, :] = r[j].T
    return out


def run(queries, keys, values, valid_lens, trace=False):
    queries = np.asarray(queries, np.float32)
    keys = np.asarray(keys, np.float32)
    values = np.asarray(values, np.float32)
    valid_lens = np.asarray(valid_lens)
    N_list, assign = _schedule(valid_lens)
    nc, coff = _build_program(N_list)
    in_maps = _stage_inputs(queries, keys, values, valid_lens, N_list,
                            assign, coff)
    res = run_bass_kernel_spmd(nc, in_maps, list(range(N_CORES)),
                               trace=trace)
    return _gather(res.results, assign), res


def kernel(queries, keys, values, valid_lens):
    out, _ = run(queries, keys, values, valid_lens)
    return out


# revision 17
# speedup vs baseline: 1.1687x; 1.1687x over previous
"""Masked dot-product attention (B=16, Lq=Lk=2048, d=64) on 8 TRN2 NeuronCores.

Distribution
------------
Attention rows are independent, so work is split into 64 units = (batch,
512-query chunk). Unit cost = ceil(valid_len/128) k-tiles; fully-masked
k-tiles contribute exactly zero and are skipped. Units are sorted by cost
(ascending) and snake-assigned to 8 slots x 8 cores; each slot's tile
count is the max within the slot, so all 8 cores run ONE shared SPMD
program (per-core differences live only in the staged data).

Device math per unit (S^T formulation; softmax over the partition axis):
    s_t[k, q]  = (K^T_t weights) @ Q^T           (PE, bf16 in / f32 PSUM)
    p_t[k, q]  = exp(0.125 * s_t)                (ACT, PSUM->SBUF, bf16)
    pv[v, q]  += V'_t^T @ p_t                    (PE, accumulate over t)
where V'_t = [V rows | ones], with rows >= valid_len zeroed on the host —
this applies the key mask AND computes the softmax denominator l = pv[64]
inside the same matmul. No row-max subtraction is needed: scores are
O(+-10) (exact softmax shift-invariance; masked lanes match the
reference's exp(-1e6)->0). valid_len == 0 reproduces jax's uniform
softmax by zeroing Q (s = 0 -> p = 1) and leaving V' unmasked.

The device returns RAW [pv[0:64] | l] per slot; the final o = pv/l
division happens vectorized on the host. This removes the entire device
epilogue (reciprocal + cross-partition broadcast chains) that previously
dominated the kernel tail.

Performance notes (measured on this axon-tunneled TRN2):
- The PE streams 1 column/cycle regardless of dtype; the clock ramps
  ~0.8 -> 1.3 GHz once HAM grants full credits, then halves if the
  kernel runs long. Shorter wall time = more of the kernel at full clock.
- kv, Q^T and p are staged/produced in bf16: halves HBM traffic (DMA
  prefetch completes by ~20us, which empirically pulls the HAM full-
  clock grant much earlier) at ~3.3e-3 l2 rel err (gate is 2e-2).
  PSUM accumulation and the staged output stay f32.
- S matmuls are K=64 (half the PE rows), so tiles are processed in
  PAIRS: even tile's K^T sits at SBUF partitions 0:64, odd tile's at
  64:127 (host staging), Q^T is loaded twice into both partition halves
  (one DRAM copy, two DMAs); the pair's two matmuls run concurrently in
  disjoint PE row halves (~494ns/pair vs ~780ns serial). GROUP=2 makes
  EVERY full group a pair (the old GROUP=3 left one unpaired tile per
  group).
- kv is staged as 4-tile chunks [V'0|V'1|V'2|V'3|K01|K23] with each
  pair's K^T packed into one fully-used [128,128] block — no zero
  padding in the K region (the old layout DMA'd 33% zeros), 2KB+
  partition lines, and only ~15 kv DMA issues per core.
- DMA issues are spread across queues: kv on Sync, Q^T on Vector,
  outputs on GpSimd — issue serialization (~0.6us/DMA) never gates
  the first matmul.
- The PE instruction queue is in-order, so PV matmuls of pair-group g
  are emitted after the S matmuls of group g+2 (two-group software
  pipeline); the PE never stalls on the exp.
- PSUM budget: s tiles 2 banks x 3 bufs + pv 1 bank x 2 bufs = 8 banks.
- Every TPB instruction may carry at most ONE sync wait on this walrus;
  split_multi_waits() post-processes the scheduled program into
  single-wait form with wait-carrying NoOps.
"""
import numpy as np
import ml_dtypes

import concourse.bass as bass
import concourse.mybir as mybir
import concourse.tile as tile
from concourse.bass_utils import run_bass_kernel_spmd


def split_multi_waits(nc):
    """TRN2 TPB instructions encode a single sync-wait slot. Tile's
    add_semaphores can emit several waits on one instruction (and the
    kernel-tail drain aggregates one per live proc), which walrus rejects
    ("Too many sync wait commands"). Rewrite every instruction carrying
    k>1 waits into (k-1) same-engine NoOps carrying one wait each."""
    for fn in nc.m.functions:
        for bb in fn.blocks:
            new = []
            for inst in bb.instructions:
                si = inst.sync_info
                ow = list(si.on_wait) if si else []
                if len(ow) > 1:
                    for jj, w in enumerate(ow[:-1]):
                        nop = mybir.InstNoOp(
                            name=f"{inst.name}_sw{jj}", ins=[], outs=[])
                        nop.engine = inst.engine
                        nop.sync_info = mybir.SyncInfo(
                            on_wait=[w], on_update=[])
                        new.append(nop)
                    inst.sync_info = mybir.SyncInfo(
                        on_wait=[ow[-1]], on_update=list(si.on_update))
                new.append(inst)
            bb.instructions = new

F32 = mybir.dt.float32
F32R = mybir.dt.float32r
BF16 = mybir.dt.bfloat16

B, L, D = 16, 2048, 64
QC = 512                 # query-chunk (free dim of both matmuls)
NQCHUNK = L // QC        # 4 chunks per batch
KT = 128                 # k rows per tile
N_CORES = 8
N_SLOTS = (B * NQCHUNK) // N_CORES   # 8 units per core
GROUP = 2                # k-tiles per pair-group (1 PSUM s-tile = 2 banks)
CHUNK = 4                # k-tiles per kv DMA chunk (2 pair-groups)
W_CHUNK = CHUNK * 65 + (CHUNK // 2) * 128   # 516 f32 per partition row


def _chunk_width(u):
    """Staged f32 width of a kv chunk holding u (1..4) tiles: CHUNK V'
    blocks of 65 (trailing ones zero-padded) then ceil(u/2) packed K^T
    blocks of 128 at the fixed CHUNK*65 base."""
    return CHUNK * 65 + ((u + 1) // 2) * 128


def _schedule(valid_lens):
    """Snake-assign 64 units to 8 slots x 8 cores. Returns (N_list, assign)
    where assign[core][slot] = (batch, qchunk) and N_list[slot] = tile
    count every core runs for that slot."""
    evl = np.where(valid_lens > 0, valid_lens, L).astype(np.int64)
    cost = np.ceil(evl / KT).astype(np.int64)        # per batch
    units = [(int(cost[b]), b, qc) for b in range(B) for qc in range(NQCHUNK)]
    units.sort(key=lambda t: (t[0], t[1], t[2]))
    N_list = []
    assign = [[None] * N_SLOTS for _ in range(N_CORES)]
    for j in range(N_SLOTS):
        grp = units[j * N_CORES:(j + 1) * N_CORES]
        N_list.append(grp[-1][0])
        for c in range(N_CORES):
            _, b, qc = grp[c]
            assign[c][j] = (b, qc)
    return N_list, assign


_PROGRAM_CACHE = {}


def _build_program(N_list):
    key = tuple(N_list)
    if key in _PROGRAM_CACHE:
        return _PROGRAM_CACHE[key]
    # kv staged chunk-major: chunk = up to 4 tiles in one DMA
    n_chunks = [int(np.ceil(n / CHUNK)) for n in N_list]
    TC = int(sum(n_chunks))
    coff = [0]
    for g in n_chunks:
        coff.append(coff[-1] + g)

    nc = bass.Bass()
    kv_d = nc.declare_dram_parameter("kv", [TC, KT, W_CHUNK], BF16,
                                     isOutput=False)
    qT_d = nc.declare_dram_parameter("qT", [N_SLOTS, D, QC], BF16,
                                     isOutput=False)
    o_d = nc.declare_dram_parameter("o", [N_SLOTS, 65, QC], F32,
                                    isOutput=True)

    with tile.TileContext(nc) as tc:
        with (
            tc.tile_pool(name="kv_pool", bufs=15) as kv_pool,
            tc.tile_pool(name="q_pool", bufs=1) as q_pool,
            tc.tile_pool(name="p_pool", bufs=5) as p_pool,
            tc.tile_pool(name="ep_pool", bufs=3) as ep_pool,
            tc.tile_pool(name="warm_pool", bufs=1) as warm_pool,
            tc.tile_pool(name="s_pool", bufs=3, space="PSUM") as s_pool,
            tc.tile_pool(name="pv_pool", bufs=2, space="PSUM") as pv_pool,
        ):
            # ACT exp-table warm-up: overlap the one-time table load with
            # the first DMAs instead of stalling the first real group.
            warm = warm_pool.tile([1, 1], F32)
            nc.vector.memset(warm, 0.0)
            nc.scalar.activation(warm, warm, mybir.ActivationFunctionType.Exp)

            # All 8 units' Q^T in one resident tile. One DRAM copy, two
            # half-height DMAs per slot (both halves identical) so the S
            # pair's even/odd matmuls find Q^T in both partition halves.
            # Two parallel DMAs also double the in-flight rate for slot 0.
            qt_all = q_pool.tile([KT, N_SLOTS, QC], BF16)

            def load_qt(j):
                src = bass.AP(tensor=qT_d, offset=j * D * QC,
                              ap=[[QC, D], [1, QC]])
                nc.gpsimd.dma_start(out=qt_all[0:D, j, :], in_=src)
                nc.gpsimd.dma_start(out=qt_all[D:KT, j, :], in_=src)

            # prefetch ALL slots' Q^T upfront: the 16 small DMAs issue
            # early on the GpSimd queue (otherwise idle until epilogues),
            # front-loading HBM traffic so the DMA system goes quiet as
            # soon as possible
            for j in range(N_SLOTS):
                load_qt(j)

            # two-group-deep software pipeline: PV matmuls of pair-group g
            # are emitted after the S matmuls of group g+2, so the in-order
            # PE queue never waits on the exp.
            PIPE_DEPTH = 2
            pending = []       # [(pv, pvsl, p, t0, n, g, j), ...]

            def flush_one():
                if not pending:
                    return
                pv, pvsl, p, t0, n, g, _ = pending.pop(0)
                for i in range(g):
                    nc.tensor.matmul(pv[0:65, :], lhsT=pvsl[i],
                                     rhs=p[:, i * QC:(i + 1) * QC],
                                     start=(t0 + i == 0),
                                     stop=(t0 + i == n - 1))

            epilogues = []     # (j, pv) awaiting copy-out (after PV flush)

            def emit_epilogues():
                # a slot's copy-out may only run once every PV group of its
                # unit has been flushed (program order defines semantics);
                # it frees the PSUM accumulator bank
                while epilogues and (not pending
                                     or epilogues[0][0] < pending[0][6]):
                    j, pv = epilogues.pop(0)
                    pvc = ep_pool.tile([65, QC], F32, tag="pvc")
                    nc.vector.tensor_copy(pvc, pv[0:65, :])
                    nc.gpsimd.dma_start(out=o_d[j], in_=pvc)

            for j in range(N_SLOTS):
                n = N_list[j]
                pv = pv_pool.tile([KT, QC], F32, tag="pv")
                t = 0
                kvc = None
                for t in range(0, n, GROUP):
                    g = min(GROUP, n - t)
                    if t % CHUNK == 0:
                        u = min(CHUNK, n - t)
                        w = _chunk_width(u)
                        cidx = coff[j] + t // CHUNK
                        kvc = kv_pool.tile([KT, W_CHUNK], BF16, tag="kv")
                        nc.sync.dma_start(out=kvc[:, 0:w],
                                          in_=kv_d[cidx][:, 0:w])
                        kbase = CHUNK * 65
                    s = s_pool.tile([KT, GROUP * QC], F32, tag="s")
                    pvsl = []
                    for i in range(g):
                        ic = (t % CHUNK) + i          # tile index in chunk
                        lo = D * (ic % 2)
                        ko = kbase + (ic // 2) * 128  # packed K^T block
                        nc.tensor.matmul(s[:, i * QC:(i + 1) * QC],
                                         lhsT=kvc[lo:lo + D, ko:ko + 128],
                                         rhs=qt_all[lo:lo + D, j, :],
                                         start=True, stop=True)
                        pvsl.append(kvc[:, ic * 65:(ic + 1) * 65])
                    p = p_pool.tile([KT, GROUP * QC], BF16, tag="p")
                    nc.scalar.activation(p[:, 0:g * QC], s[:, 0:g * QC],
                                         mybir.ActivationFunctionType.Exp,
                                         scale=0.125)
                    if len(pending) >= PIPE_DEPTH:
                        flush_one()
                        emit_epilogues()
                    pending.append((pv, pvsl, p, t, n, g, j))
                epilogues.append((j, pv))
            while pending:
                flush_one()
                emit_epilogues()
            emit_epilogues()

    split_multi_waits(nc)
    _PROGRAM_CACHE[key] = (nc, coff)
    return nc, coff


def _stage_inputs(queries, keys, values, valid_lens, N_list, assign, coff):
    evl = np.where(valid_lens > 0, valid_lens, L).astype(np.int64)
    zero_q = valid_lens <= 0
    TC = coff[-1]

    # Per-batch precomputed host tensors
    kTT = np.ascontiguousarray(keys.transpose(0, 2, 1))        # [B, D, L]
    vmask = (np.arange(L)[None, :] < evl[:, None])             # [B, L]
    vp = np.concatenate(
        [values, np.ones((B, L, 1), np.float32)], axis=2)      # [B, L, 65]
    vp = vp * vmask[:, :, None].astype(np.float32)

    in_maps = []
    for c in range(N_CORES):
        # chunk-major: kv[ch] = [V'0|V'1|V'2|V'3|K01|K23]; pair (2i,2i+1)
        # K^T packed into one [128,128] block (even rows 0:64, odd 64:128)
        kv = np.zeros((TC, KT, W_CHUNK), ml_dtypes.bfloat16)
        qT = np.zeros((N_SLOTS, D, QC), ml_dtypes.bfloat16)
        for j in range(N_SLOTS):
            b, qc = assign[c][j]
            n_real = int(np.ceil(evl[b] / KT))
            if not zero_q[b]:
                qT[j] = queries[b, qc * QC:(qc + 1) * QC, :].T
            n = min(n_real, N_list[j])
            nch = coff[j + 1] - coff[j]
            ntp = nch * CHUNK                      # padded tile count
            vt = np.zeros((ntp, KT, 65), np.float32)
            kt = np.zeros((ntp, D, KT), np.float32)
            vt[0:n] = vp[b, 0:n * KT].reshape(n, KT, 65)
            kt[0:n] = kTT[b, :, 0:n * KT].reshape(D, n, KT).transpose(1, 0, 2)
            sl = slice(coff[j], coff[j + 1])
            # V' blocks at i*65
            kv[sl, :, 0:CHUNK * 65] = (
                vt.reshape(nch, CHUNK, KT, 65)
                  .transpose(0, 2, 1, 3).reshape(nch, KT, CHUNK * 65))
            # packed K^T: pair p of chunk -> cols CHUNK*65 + p*128
            ktg = kt.reshape(nch, CHUNK // 2, 2, D, KT)
            kb = CHUNK * 65
            for pidx in range(CHUNK // 2):
                kv[sl, 0:D, kb + pidx * 128:kb + pidx * 128 + KT] = \
                    ktg[:, pidx, 0]
                kv[sl, D:, kb + pidx * 128:kb + pidx * 128 + KT] = \
                    ktg[:, pidx, 1]
        in_maps.append({"kv": kv, "qT": qT})
    return in_maps


def _gather(results, assign):
    out = np.empty((B, L, D), np.float32)
    for c in range(N_CORES):
        o = results[c]["o"]                       # [N_SLOTS, 65, QC]
        num = o[:, 0:D, :]
        den = o[:, D:D + 1, :]
        r = num / den                             # [N_SLOTS, D, QC]
        for j in range(N_SLOTS):
            b, qc = assign[c][j]
            out[b, qc * QC:(qc + 1) * QC, :] = r[j].T
    return out


def run(queries, keys, values, valid_lens, trace=False):
    queries = np.asarray(queries, np.float32)
    keys = np.asarray(keys, np.float32)
    values = np.asarray(values, np.float32)
    valid_lens = np.asarray(valid_lens)
    N_list, assign = _schedule(valid_lens)
    nc, coff = _build_program(N_list)
    in_maps = _stage_inputs(queries, keys, values, valid_lens, N_list,
                            assign, coff)
    res = run_bass_kernel_spmd(nc, in_maps, list(range(N_CORES)),
                               trace=trace)
    return _gather(res.results, assign), res


def kernel(queries, keys, values, valid_lens):
    out, _ = run(queries, keys, values, valid_lens)
    return out
